# revision 28
# baseline (speedup 1.0000x reference)
"""SPGAT (single-layer GAT, batch=1) Trainium2 kernel, 8-core row-parallel.

Math (reference):
    Wh  = inputs @ W                          [N, D]
    f1  = Wh @ a1, f2 = Wh @ a2               [N, 1]
    e   = leaky_relu(f1 + f2.T, 0.2)          [N, N]
    att = softmax(where(adj > 0, e, -inf))    [N, N]
    out = relu(att @ Wh)                      [N, D]

Key reformulations:
  * Masked softmax == multiply exp(e) by the 0/1 adjacency and normalize by
    the masked row-sum (exact; adj is 0/1).  Normalization is deferred past
    the aggregation matmul: out_r = relu((P @ Wh)_r / s_r) with
    P = adj * exp(e); s_r comes free from a ones-column appended to Wh.
  * exp is monotone, so exp(leaky_relu(s)) = max(exp(s), exp(0.2 s)), and
    exp(f1 + f2) factorizes rank-1.  Each softmax row is scale-invariant, so
    divide row r by exp(0.2 f1[r]):
        P'[c, r] = adj[r, c] * max(b2[c], g[r] * b1[c]),
        g = exp(0.8 f1), b1 = exp(f2), b2 = exp(0.2 f2),
    which changes neither att nor the output.  No dense transcendentals and
    no rank-1 A/B tiles remain: per [128, 2048] tile-pair the elementwise
    work is one dual-scalar tensor_scalar ((g*b1c) max b2c, VectorE 4x mode)
    per half plus one mask multiply (2x tensor_tensor) — or, on a quarter of
    the pairs for engine balance, ScalarE Relu(g*b1c - b2c) halves followed
    by a fused (t + b2c) * adj scalar_tensor_tensor on VectorE.
  * Everything N x N is produced directly in transposed [c, r] layout so the
    PE contraction (over c) needs no on-device transposes: 16 lhsT slices
    per tile-pair feed 8 PSUM accumulators [128, D+1] (one per row block).

Sharding: rows split 1024/core over 8 cores; the per-core adj^T column block
is host-prepared (transpose + cast to bf16 — exact for a 0/1 mask).  The
O(N D^2) projections (Wh = inputs@W and the rank-1 f1/f2/exp vectors, ~3% of
FLOPs) are host prep, replicated to all cores; all O(N^2) attention work
(34 GFLOP) runs on-device.  No collectives are needed.
"""

import os
import sys

import numpy as np

try:
    import concourse.bass as bass  # noqa: F401
except Exception:  # pragma: no cover - grading env fallback
    for p in ("/opt/trn_rl_repo", "/root/.axon_site/_ro/trn_rl_repo"):
        if os.path.isdir(p) and p not in sys.path:
            sys.path.insert(0, p)
    import concourse.bass as bass  # noqa: F401

import ml_dtypes

import concourse.tile as tile
from concourse import bacc, bass_utils, mybir

N = 8192
D = 256
NCORES = 8
R = N // NCORES  # rows per core = 1024
RT = R // 128    # r tiles per core = 8
CT = N // 128    # c tiles = 64
CP = CT // 2     # c tile pairs = 32
ALPHA = 0.2

F32 = mybir.dt.float32
BF16 = mybir.dt.bfloat16
BF16_NP = ml_dtypes.bfloat16

AF = mybir.ActivationFunctionType
OP = mybir.AluOpType


def act_pair(u):
    # pairs whose t0 halves run on ScalarE (engine load balance)
    return u % 5 in (1, 3)


def build_nc():
    nc = bacc.Bacc("TRN2", target_bir_lowering=False, debug=False,
                   num_devices=NCORES)

    # paired layouts: one 2D DMA per c-chunk pair (4 KB / 1 KB lines)
    adjT_d = nc.dram_tensor("adjt", [CP * 128, 2 * R], BF16,
                            kind="ExternalInput")
    whp_d = nc.dram_tensor("whp", [CP * 128, 2 * (D + 1)], BF16,
                           kind="ExternalInput")
    gb_d = nc.dram_tensor("gb", [128, R], BF16, kind="ExternalInput")
    bv_d = nc.dram_tensor("bv", [128, 3, CT], F32, kind="ExternalInput")
    out_d = nc.dram_tensor("out", [R, D], F32, kind="ExternalOutput")

    with tile.TileContext(nc) as tc:
        with (
            tc.tile_pool(name="const", bufs=1) as cpool,
            tc.tile_pool(name="whp", bufs=8) as whp_pool,
            tc.tile_pool(name="work", bufs=8) as work,
            tc.tile_pool(name="deep", bufs=10) as deep,
            tc.tile_pool(name="fin", bufs=3) as fin,
            tc.tile_pool(name="ps", bufs=8, space=bass.MemorySpace.PSUM) as ps,
        ):
            # ---------------- constants ----------------
            gb = cpool.tile([128, R], BF16, name="gb")  # exp(0.8 f1[r])
            nc.sync.dma_start(gb[:], gb_d[:, :])        # host pre-broadcast
            bv = cpool.tile([128, 3, CT], F32, name="bv")  # b1 | b2 | -b2
            nc.sync.dma_start(bv[:], bv_d[:, :, :])
            b1c = bv[:, 0, :]
            b2c = bv[:, 1, :]
            nb2c = bv[:, 2, :]

            # ---------------- accumulators (live across the c loop) -----
            accs = [ps.tile([128, D + 1], F32, tag="ps", name=f"acc{j}")
                    for j in range(RT)]

            # PE HAM warm-up: ~48 dummy matmuls on arbitrary resident data,
            # discarded by the start=True of the first real accumulation.
            for i in range(48):
                nc.tensor.matmul(accs[0][:, :], gb[:, 0:128], gb[:, 0:257],
                                 start=(i == 0), stop=(i == 47))

            # whp tiles all preloaded up front via the gpsimd SWDGE queues
            # (tiny: 4.2 MB total), leaving the sync queues to the adjacency
            # stream.
            whp = []
            for u in range(CP):
                wt = whp_pool.tile([128, 2, D + 1], BF16, tag="whp",
                                   name=f"whp{u}")
                nc.gpsimd.dma_start(
                    wt[:, :, :], whp_d[u * 128:(u + 1) * 128, :])
                whp.append(wt)

            # ------------- main loop over pairs of c chunks -------------
            # t0[c, r] = max(g[r]*b1[c], b2[c]) == exp(lrelu(f1+f2))/exp(.2f1)
            # P[c, r] = adj[r, c] * t0[c, r]
            for u in range(CP):
                adj_sb = deep.tile([128, 2, R], BF16, tag="adj",
                                   name=f"adj{u}")
                wt = whp[u]
                t0 = work.tile([128, 2, R], BF16, tag="t0", name=f"t0{u}")
                p_sb = deep.tile([128, 2, R], BF16, tag="p", name=f"p{u}")
                nc.sync.dma_start(adj_sb[:, :, :],
                                  adjT_d[u * 128:(u + 1) * 128, :])
                for h in range(2):
                    t = 2 * u + h
                    if act_pair(u):
                        # both t0 stages on ScalarE:
                        # t0 = relu(g*b1c - b2c) + b2c == max(g*b1c, b2c)
                        tr = work.tile([128, R], BF16, tag="tr",
                                       name=f"tr{u}_{h}")
                        nc.scalar.activation(tr[:], gb[:], AF.Relu,
                                             bias=nb2c[:, t:t + 1],
                                             scale=b1c[:, t:t + 1])
                        nc.scalar.activation(t0[:, h, :], tr[:], AF.Identity,
                                             bias=b2c[:, t:t + 1], scale=1.0)
                    else:
                        nc.vector.tensor_scalar(t0[:, h, :], gb[:],
                                                b1c[:, t:t + 1],
                                                b2c[:, t:t + 1],
                                                OP.mult, OP.max)
                nc.vector.tensor_mul(p_sb[:, :, :], t0[:, :, :],
                                     adj_sb[:, :, :])

                for h in range(2):
                    t = 2 * u + h
                    for j in range(RT):
                        nc.tensor.matmul(
                            accs[j][:, :],
                            p_sb[:, h, j * 128:(j + 1) * 128],
                            wt[:, h, :],
                            start=(t == 0), stop=(t == CT - 1),
                        )

            # ---------------- normalize + relu + store ----------------
            o_all = fin.tile([128, RT, D], F32, name="o_all")
            for j in range(RT):
                rec = fin.tile([128, 1], F32, tag="rec", name=f"rec{j}")
                nc.vector.reciprocal(rec[:], accs[j][:, D:D + 1])
                if j % 2 == 0:
                    # relu(acc * rec) via DVE dual-op tensor_scalar
                    nc.vector.tensor_scalar(o_all[:, j, :], accs[j][:, 0:D],
                                            rec[:], 0.0, OP.mult, OP.max)
                else:
                    nc.scalar.activation(o_all[:, j, :], accs[j][:, 0:D],
                                         AF.Relu, bias=0.0, scale=rec[:])
            # single batched store: out[j*128+p, d] <- o_all[p, j, d]
            out_ap = out_d.ap().rearrange("(j p) d -> p j d", p=128)
            nc.sync.dma_start(out_ap, o_all[:, :, :])

    nc.compile()
    return nc


_CACHE = {}


def _get_nc():
    if "nc" not in _CACHE:
        _CACHE["nc"] = build_nc()
    return _CACHE["nc"]


def make_in_maps(inputs, adj, W, a1, a2):
    inputs = np.asarray(inputs, dtype=np.float32)
    adj = np.asarray(adj, dtype=np.float32)
    W = np.asarray(W, dtype=np.float32)
    a1 = np.asarray(a1, dtype=np.float32)
    a2 = np.asarray(a2, dtype=np.float32)

    # projections (~3% of FLOPs) on host, replicated to all cores
    Wh = inputs @ W
    f1 = (Wh @ a1).reshape(N).astype(np.float32)
    f2 = (Wh @ a2).reshape(N).astype(np.float32)
    whp = np.concatenate(
        [Wh, np.ones((N, 1), np.float32)], axis=1).astype(BF16_NP)
    # paired tile layout: row u*128+p holds chunks 2u and 2u+1 side by side
    whp_p = np.ascontiguousarray(
        whp.reshape(CP, 2, 128, D + 1).transpose(0, 2, 1, 3)
           .reshape(CP * 128, 2 * (D + 1)))

    b1 = np.ascontiguousarray(np.exp(f2).reshape(CT, 128).T)         # [128,CT]
    b2 = np.ascontiguousarray(np.exp(ALPHA * f2).reshape(CT, 128).T)
    bv = np.ascontiguousarray(
        np.stack([b1, b2, -b2], axis=1))  # [128, 3, CT]

    adj_bf = adj.astype(BF16_NP)  # exact: adj entries are 0/1
    in_maps = []
    for k in range(NCORES):
        r0, r1 = k * R, (k + 1) * R
        adjT_k = np.ascontiguousarray(adj_bf[r0:r1, :].T)  # [N, R]
        adjT_p = np.ascontiguousarray(
            adjT_k.reshape(CP, 2, 128, R).transpose(0, 2, 1, 3)
                  .reshape(CP * 128, 2 * R))
        in_maps.append({
            "adjt": adjT_p,
            "whp": whp_p,
            "gb": np.ascontiguousarray(np.broadcast_to(
                np.exp((1.0 - ALPHA) * f1[r0:r1]).reshape(1, R)
                .astype(BF16_NP), (128, R))),
            "bv": bv,
        })
    return in_maps


def run(in_maps, trace=False):
    nc = _get_nc()
    res = bass_utils.run_bass_kernel_spmd(
        nc, [dict(m) for m in in_maps], core_ids=list(range(NCORES)),
        trace=trace,
    )
    out = np.concatenate([res.results[k]["out"] for k in range(NCORES)],
                         axis=0)
    return out, res


def kernel(inputs, adj, cmt_weight, W, a1, a2):
    in_maps = make_in_maps(inputs, adj, W, a1, a2)
    out, _ = run(in_maps, trace=False)
    return out.astype(np.float32)
